# revision 28
# baseline (speedup 1.0000x reference)
"""Trainium2 Bass kernel for the RNN-T JointNetwork problem.

Computes log_softmax(tanh(cat(enc, pred)) @ W.T + b) over the vocab dim
for logits of shape [B=4, T=200, U=50, V=1024], fp32.

Strategy (data-parallel over the 800 flattened (b,t) rows, 100 per core):

  Key identity: sum_v exp(e[t,v] + p[u,v]) = exp(e[t,:]) . exp(p[u,:]),
  so the log-softmax denominator for ALL 5000x1024 logits per core is a
  single [100,1024] x [1024,50] matmul over exp'd projections -- no exp
  over the big tensor.

  The joint moving operands M_A/M_B hold pred_b at partitions 0-49 and an
  enc_p half at partitions 64+ (PE tile_position=(0,64)), so the setup
  matmuls write PSUM directly in M layout and plain lane-aligned engine
  copies produce M -- no partition-crossing SBUF-SBUF DMAs anywhere.

  setup (per core, on device):
    tpT/teT = tanh(predT/encT)                      (ACT)
    MA_ps[0:50]    = tpT.T @ WpT + b    (PE)
    MA_ps[64:116]  = teT[:,0:52].T @ WeT
    MB_ps[64:114]  = teT[:,50:100].T @ WeT
    m_a = copy(MA_ps), m_b = [copy(m_a[0:50]); copy(MB_ps)]  (bf16)
    ep  = exp(MA_ps[0:50]), eeA/eeB = exp(enc halves)        (ACT, bf16)
    epT/eeT = PE transposes (identity blocks at the right partitions)
    S[t,u] = ee[t,:] . ep[u,:]  (PE, 8 matmuls);  nlse = -ln(S)
    nlse --DMA--> DRAM --DMA--> nlse_rows [128,40] (row-major flatten)
  main loop over 40 row-tiles of 128 rows (row r = t*50 + u), each tile
  split into two 512-vocab halves (single-bank psum tiles):
    x    = comb_k.T @ M     (PE: ONE one-hot matmul per half; comb rows
           0-49 u one-hot, rows 64+ t one-hot; tile 19 uses the 2 extra
           enc rows carried in M_A)
    tiles 0..HYB-1: local lse (ACT exp+accum per half + ln) so output
           DMA starts before the nlse round trip lands.
    tiles HYB..: half a: DVE tensor_scalar_add(x, nlse_rows[:,k]);
                 half b: ACT Identity-with-bias. (bf16 out)
    DMA out per PAIR of tiles -> DRAM (gpsimd/scalar/sync rotation)

  Output is written bf16 (halves the 20.5MB/core store) and upcast to
  fp32 on the host; bf16 rounding is ~0.4% relative, far inside the
  tolerance.
"""

import numpy as np

import concourse.bass as bass
import concourse.bacc as bacc
import concourse.tile as tile
from concourse import mybir
from concourse.bass_utils import run_bass_kernel_spmd

# Problem shapes (hardcoded per contract).
B, T, U, D, V = 4, 200, 50, 512, 1024
N_CORES = 8
BT = B * T                     # 800 flattened (b,t) rows
TPC = BT // N_CORES            # 100 (b,t) rows per core
ROWS = TPC * U                 # 5000 output rows per core
P = 128
NT = (ROWS + P - 1) // P       # 40 row-tiles per core
HV = 512                       # vocab half (psum-bank limit: 512 f32)
DC = D // P                    # 4 contraction chunks of 128 for D=512
VC = V // P                    # 8 vocab chunks of 128 (transposes / S)
KSPLIT = 19                    # tile 19 straddles the M_A/M_B split
EB = 64                        # partition base of the enc block in M
MRA = EB + 52                  # 116 M_A rows (enc rows 0-51 at 64-115)
MRB = EB + 50                  # 114 M_B rows (enc rows 50-99 at 64-113)
HYB = 7                        # tiles 0..6 compute lse locally (latency)

f32 = mybir.dt.float32
bf16 = mybir.dt.bfloat16

TRACE = False
LAST_RESULT = None

_CACHE = {}


def _patch_act_tables():
    """Pin Exp/Ln to the one table set containing both, so the activation
    table-load pass never alternates sets mid-kernel."""
    if getattr(bacc, "_joint_act_patch", False):
        return
    orig = bacc.get_activation_tables

    def patched(arch):
        t = dict(orig(arch))
        keep = "natural_log_exp_and_others"
        drop = {mybir.ActivationFunctionType.Exp, mybir.ActivationFunctionType.Ln}
        for name, fns in t.items():
            if name != keep:
                t[name] = set(fns) - drop
        return t

    bacc.get_activation_tables = patched
    bacc._joint_act_patch = True


def _build_indicators():
    """Per-row-tile one-hot stationary operands [116, NT, 128], shared by
    all cores.

    comb[:, k, :]: rows 0-49 are the u one-hot (row u has a 1 in column c
    iff u(r)=u for r=128k+c), rows 64.. the t one-hot at row 64+t-off
    with off=0 for k<=19 and off=50 for k>19.  Tile 19 spans t=48..51 and
    uses rows 112-115 (M_A carries enc rows 50,51 at partitions 114,115).
    Columns for rows >= ROWS (tail of the last tile) are all-zero.
    """
    comb = np.zeros((NT, MRA, P), dtype=np.float32)
    for k in range(NT):
        r = np.arange(k * P, min((k + 1) * P, ROWS))
        c = np.arange(len(r))
        off = 50 if k > KSPLIT else 0
        comb[k, (r % U)[c], c] = 1.0
        comb[k, EB + (r // U)[c] - off, c] = 1.0
    return np.ascontiguousarray(comb.transpose(1, 0, 2))


def _build_program():
    _patch_act_tables()
    nc = bacc.Bacc("TRN2", target_bir_lowering=False, debug=False,
                   num_devices=N_CORES)

    # encT/predT arrive pre-chunked [128, DC, *] so each partition's DMA
    # read is one contiguous run (128 descriptors, not 512).
    encT = nc.dram_tensor("encT", [P, DC, TPC], f32, kind="ExternalInput")
    predT = nc.dram_tensor("predT", [P, DC, U], f32, kind="ExternalInput")
    wT = nc.dram_tensor("wT", [2 * D, V], bf16, kind="ExternalInput")
    bias = nc.dram_tensor("bias", [V], bf16, kind="ExternalInput")
    comb = nc.dram_tensor("comb", [MRA, NT, P], bf16, kind="ExternalInput")
    ident = nc.dram_tensor("ident", [P, P], bf16, kind="ExternalInput")
    # identity block at partitions 64-127 (for the enc-half transposes)
    ident2 = nc.dram_tensor("ident2", [P, EB], bf16, kind="ExternalInput")
    lse_scr = nc.dram_tensor("lse_scr", [NT * P], f32, kind="Internal")
    out = nc.dram_tensor("out", [ROWS, V], bf16, kind="ExternalOutput")

    Act = mybir.ActivationFunctionType
    HALF = (slice(0, HV), slice(HV, V))

    with tile.TileContext(nc) as tc:
        with (
            tc.tile_pool(name="consts", bufs=1) as consts,
            tc.tile_pool(name="psum", bufs=6, space=bass.MemorySpace.PSUM) as psum,
            tc.tile_pool(name="outs", bufs=6) as outs,
            tc.tile_pool(name="scratch", bufs=2) as scratch,
            tc.tile_pool(name="small", bufs=8) as small,
        ):
            # ---- input DMAs, spread over the three issue queues; the
            #      pred-chain inputs (predT, Wp) come first ----
            wt_sb = consts.tile([P, 2 * DC, V], bf16)
            wT_r = wT.ap().rearrange("(c p) v -> p c v", p=P)
            # sync: predT (gates the first tanh), Wp 0, We 0-1, round trip,
            # every 3rd output pair
            predT_sb = consts.tile([P, DC, U], f32)
            nc.sync.dma_start(out=predT_sb[:], in_=predT.ap())
            nc.sync.dma_start(out=wt_sb[:, DC + 0, :], in_=wT_r[:, DC + 0, :])
            nc.sync.dma_start(out=wt_sb[:, 0, :], in_=wT_r[:, 0, :])
            nc.sync.dma_start(out=wt_sb[:, 1, :], in_=wT_r[:, 1, :])
            # scalar: ALL its DMA issues precede engine work (the queue is
            # the ACT sequencer; engine waits would starve them)
            nc.scalar.dma_start(out=wt_sb[:, DC + 1, :], in_=wT_r[:, DC + 1, :])
            nc.scalar.dma_start(out=wt_sb[:, DC + 3, :], in_=wT_r[:, DC + 3, :])
            nc.scalar.dma_start(out=wt_sb[:, 2, :], in_=wT_r[:, 2, :])
            nc.scalar.dma_start(out=wt_sb[:, 3, :], in_=wT_r[:, 3, :])
            # gpsimd: encT, bias, Wp 2, identities, indicators
            encT_sb = consts.tile([P, DC, TPC], f32)
            nc.gpsimd.dma_start(out=encT_sb[:], in_=encT.ap())
            b_sb = consts.tile([1, V], bf16)
            nc.gpsimd.dma_start(out=b_sb[:], in_=bias.ap().rearrange(
                "(p v) -> p v", p=1))
            nc.gpsimd.dma_start(out=wt_sb[:, DC + 2, :], in_=wT_r[:, DC + 2, :])
            ident_sb = consts.tile([P, P], bf16)
            nc.gpsimd.dma_start(out=ident_sb[:], in_=ident.ap())
            ident2_sb = consts.tile([P, EB], bf16)
            nc.gpsimd.dma_start(out=ident2_sb[:], in_=ident2.ap())
            comb_sb = consts.tile([MRA, NT, P], bf16)
            nc.gpsimd.dma_start(out=comb_sb[:, :6, :], in_=comb.ap()[:, :6, :])
            nc.gpsimd.dma_start(out=comb_sb[:, 6:, :], in_=comb.ap()[:, 6:, :])
            ones_u = consts.tile([1, U], bf16)
            nc.vector.memset(ones_u[:], 1.0)

            # ---- PE warmup: a few junk matmuls lift the tensor engine
            #      out of its cold p-state before the real matmuls ----
            ones_row = consts.tile([1, P], bf16)
            nc.vector.memset(ones_row[:], 1.0)
            junk_in = consts.tile([1, HV], bf16)
            nc.vector.memset(junk_in[:], 1.0)
            # M tiles: zero partitions 32-63 first (rows 50-63 are never
            # written; a NaN there would poison the 0-weighted matmul)
            m_a = consts.tile([MRA, V], bf16)
            m_b = consts.tile([MRB, V], bf16)
            nc.vector.memset(m_a[32:EB, :], 0.0)
            nc.vector.memset(m_b[32:EB, :], 0.0)
            for i in range(4):
                jp = psum.tile([P, HV], f32, tag="x", name=f"junk{i}")
                nc.tensor.matmul(jp[:], ones_row[:], junk_in[:],
                                 start=True, stop=True)

            # ---- tanh ----
            tpT = consts.tile([P, DC, U], bf16)
            nc.scalar.activation(tpT[:], predT_sb[:], Act.Tanh)
            teT = consts.tile([P, DC, TPC], bf16)
            nc.scalar.activation(teT[:], encT_sb[:], Act.Tanh)
            # dummy exp: pull the Exp/Ln activation-table load off the
            # critical path (it costs ~1.5us on first use)
            dummy_e = consts.tile([1, U], f32)
            nc.scalar.activation(dummy_e[:], ones_u[:], Act.Exp)

            # ---- setup matmuls straight into M-layout psum ----
            # contraction chunks consumed in expected DMA-arrival order
            PORD = (1, 0, 3, 2)
            EORD = (0, 2, 1, 3)
            ma_ps = []
            for h in range(2):
                ph = psum.tile([P, HV], f32, tag="x", name=f"ma_ps{h}")
                for i, c in enumerate(PORD):
                    nc.tensor.matmul(ph[:U, :], tpT[:, c, :],
                                     wt_sb[:, DC + c, HALF[h]],
                                     start=(i == 0), stop=False)
                nc.tensor.matmul(ph[:U, :], ones_u[:], b_sb[:, HALF[h]],
                                 start=False, stop=True)
                ma_ps.append(ph)
            for h in range(2):
                ph = ma_ps[h]
                for i, c in enumerate(EORD):
                    nc.tensor.matmul(ph[EB:MRA, :], teT[:, c, 0:52],
                                     wt_sb[:, c, HALF[h]],
                                     start=(i == 0), stop=(i == DC - 1))
            mb_ps = []
            for h in range(2):
                ph = psum.tile([P, HV], f32, tag="mb", bufs=2,
                               name=f"mb_ps{h}")
                for i, c in enumerate(EORD):
                    nc.tensor.matmul(ph[EB:MRB, :], teT[:, c, 50:TPC],
                                     wt_sb[:, c, HALF[h]],
                                     start=(i == 0), stop=(i == DC - 1))
                mb_ps.append(ph)

            # ---- M copies (all lane-aligned) + exp'd projections ----
            ep = consts.tile([U, V], bf16)
            eeA = consts.tile([MRA, V], bf16)
            eeB = consts.tile([MRB, V], bf16)
            for h in range(2):
                # ACT: pred block of m_a; DVE: enc block
                nc.scalar.activation(m_a[:U, HALF[h]], ma_ps[h][:U, :],
                                     Act.Copy)
                nc.vector.tensor_copy(m_a[EB:, HALF[h]], ma_ps[h][EB:MRA, :])
            for h in range(2):
                nc.scalar.activation(ep[:, HALF[h]], ma_ps[h][:U, :],
                                     Act.Exp)
                nc.scalar.activation(eeA[EB:, HALF[h]], ma_ps[h][EB:MRA, :],
                                     Act.Exp)
                nc.scalar.activation(eeB[EB:, HALF[h]], mb_ps[h][EB:MRB, :],
                                     Act.Exp)
            # m_b pred block: SBUF-SBUF lane-aligned copy on gpsimd
            for h in range(2):
                nc.gpsimd.tensor_copy(m_b[:U, HALF[h]], m_a[:U, HALF[h]])

            # ---- transposes: epT[v, u], eeT[v, t] (PE, bf16 psum) ----
            epT_ps = psum.tile([P, VC, U], bf16, tag="x")
            epT_sb = consts.tile([P, VC, U], bf16)
            for c in range(VC):
                nc.tensor.transpose(epT_ps[:, c, :],
                                    ep[:, c * P:(c + 1) * P],
                                    ident_sb[:U, :U])
                nc.scalar.activation(epT_sb[:, c, :], epT_ps[:, c, :],
                                     Act.Copy)
            eeT_ps = psum.tile([P, VC, TPC], bf16, tag="x")
            eeT_sb = consts.tile([P, VC, TPC], bf16)
            for c in range(VC):
                nc.tensor.transpose(eeT_ps[:, c, 0:50],
                                    eeA[EB:EB + 50, c * P:(c + 1) * P],
                                    ident2_sb[EB:EB + 50, 0:50])
                nc.tensor.transpose(eeT_ps[:, c, 50:TPC],
                                    eeB[EB:, c * P:(c + 1) * P],
                                    ident2_sb[EB:EB + 50, 0:50])
                nc.vector.tensor_copy(eeT_sb[:, c, :], eeT_ps[:, c, :])

            # ---- main loop; S/nlse and the m_b enc copies are emitted
            #      interleaved so early (hybrid) tiles flow first ----
            def emit_x_half(k, h):
                xh = psum.tile([P, HV], f32, tag="x", name=f"x_{k}_{h}")
                if k <= KSPLIT:
                    nc.tensor.matmul(xh[:], comb_sb[:, k, :], m_a[:, HALF[h]],
                                     start=True, stop=True)
                else:
                    nc.tensor.matmul(xh[:], comb_sb[:MRB, k, :],
                                     m_b[:, HALF[h]], start=True, stop=True)
                return xh

            o2 = None
            for k in range(NT):
                r0 = k * P
                rows = min(P, ROWS - r0)
                xa = emit_x_half(k, 0)
                xb = emit_x_half(k, 1)
                if k == 2:
                    # S[t, u] = ee[t,:] . ep[u,:];  nlse = -ln(S)
                    s_ps = psum.tile([P, HV], f32, tag="x", name="s_ps")
                    for c in range(VC):
                        nc.tensor.matmul(s_ps[:TPC, :U], eeT_sb[:, c, :],
                                         epT_sb[:, c, :],
                                         start=(c == 0), stop=(c == VC - 1))
                    lse_sb = consts.tile([TPC, U], f32)
                    nc.scalar.activation(lse_sb[:], s_ps[:TPC, :U], Act.Ln)
                    nlse = consts.tile([TPC, U], f32)
                    nc.vector.tensor_scalar_mul(nlse[:], lse_sb[:], -1.0)
                    # flatten nlse[t,u] -> per-row scalars [128, NT] via a
                    # DRAM round trip (row-major (t,u) IS row order)
                    nc.sync.dma_start(
                        out=lse_scr.ap()[:ROWS].rearrange("(t u) -> t u", u=U),
                        in_=nlse[:])
                    nlse_rows = consts.tile([P, NT], f32)
                    nc.sync.dma_start(
                        out=nlse_rows[:],
                        in_=lse_scr.ap().rearrange("(k p) -> p k", p=P))
                if k == 3:
                    # m_b enc block (not needed before tile 20)
                    for h in range(2):
                        nc.vector.tensor_copy(m_b[EB:, HALF[h]],
                                              mb_ps[h][EB:MRB, :])
                if k % 2 == 0:
                    o2 = outs.tile([P, 2, V], bf16)
                oa = o2[:, k % 2, 0:HV]
                ob = o2[:, k % 2, HV:V]
                if k < HYB:
                    # local lse: ACT exp+accum per half -> combine -> ln
                    sums_a = small.tile([P, 1], f32)
                    sums_b = small.tile([P, 1], f32)
                    escr = scratch.tile([P, V], f32)
                    nc.scalar.activation(escr[:, 0:HV], xa[:], Act.Exp,
                                         accum_out=sums_a[:])
                    nc.scalar.activation(escr[:, HV:V], xb[:], Act.Exp,
                                         accum_out=sums_b[:])
                    stot = small.tile([P, 1], f32)
                    nc.vector.scalar_tensor_tensor(
                        stot[:], sums_a[:], 0.0, sums_b[:],
                        mybir.AluOpType.add, mybir.AluOpType.add)
                    lse_h = small.tile([P, 1], f32)
                    nc.scalar.activation(lse_h[:], stot[:], Act.Ln)
                    nc.vector.tensor_scalar_sub(oa[:rows], xa[:rows],
                                                lse_h[:rows])
                    nc.vector.tensor_scalar_sub(ob[:rows], xb[:rows],
                                                lse_h[:rows])
                else:
                    nc.vector.tensor_scalar_add(oa[:rows], xa[:rows],
                                                nlse_rows[:rows, k:k + 1])
                    nc.scalar.activation(ob[:rows], xb[:rows], Act.Identity,
                                         bias=nlse_rows[:rows, k:k + 1])
                # DMA per pair; the straddling tail pair goes per-tile
                eng = (nc.gpsimd, nc.scalar, nc.sync)[(k // 2) % 3]
                if k % 2 == 1 and rows == P:
                    pr = (k - 1) * P
                    eng.dma_start(
                        out=out.ap()[pr:pr + 2 * P, :].rearrange(
                            "(two p) v -> p two v", p=P),
                        in_=o2[:])
                elif k % 2 == 1 or rows < P:
                    eng.dma_start(out=out.ap()[r0:r0 + rows, :],
                                  in_=o2[:rows, k % 2, :])
                    if k % 2 == 1:
                        eng.dma_start(out=out.ap()[r0 - P:r0, :],
                                      in_=o2[:, 0, :])

    nc.compile()
    return nc


def kernel(enc_out, pred_out, W, b):
    global LAST_RESULT
    enc_out = np.asarray(enc_out, dtype=np.float32)
    pred_out = np.asarray(pred_out, dtype=np.float32)
    W = np.asarray(W, dtype=np.float32)
    b = np.asarray(b, dtype=np.float32)

    if "nc" not in _CACHE:
        _CACHE["nc"] = _build_program()
        _CACHE["ind"] = _build_indicators()
    nc = _CACHE["nc"]
    comb = _CACHE["ind"]

    import ml_dtypes
    wT = np.ascontiguousarray(W.T).astype(ml_dtypes.bfloat16)   # [2D, V]
    enc_flat = enc_out.reshape(BT, D)                 # [800, 512]
    ident = np.eye(P, dtype=ml_dtypes.bfloat16)
    ident2 = np.zeros((P, EB), dtype=ml_dtypes.bfloat16)
    ident2[EB + np.arange(EB), np.arange(EB)] = 1
    comb_bf = comb.astype(ml_dtypes.bfloat16)
    bias_bf = b.astype(ml_dtypes.bfloat16)

    def chunkT(x):  # [n, D] -> [128, DC, n] contiguous
        return np.ascontiguousarray(
            x.T.reshape(DC, P, x.shape[0]).transpose(1, 0, 2))

    in_maps = []
    for c in range(N_CORES):
        bt0 = c * TPC
        b_idx = bt0 // T
        in_maps.append({
            "encT": chunkT(enc_flat[bt0:bt0 + TPC]),
            "predT": chunkT(pred_out[b_idx]),
            "wT": wT,
            "bias": bias_bf,
            "comb": comb_bf,
            "ident": ident,
            "ident2": ident2,
        })

    res = run_bass_kernel_spmd(nc, in_maps, core_ids=list(range(N_CORES)),
                               trace=TRACE)
    LAST_RESULT = res
    full = np.concatenate([r["out"] for r in res.results], axis=0)
    return full.reshape(B, T, U, V).astype(np.float32)


# revision 29
# speedup vs baseline: 1.1394x; 1.1394x over previous
"""Trainium2 Bass kernel for the RNN-T JointNetwork problem.

Computes log_softmax(tanh(cat(enc, pred)) @ W.T + b) over the vocab dim
for logits of shape [B=4, T=200, U=50, V=1024], fp32.

Strategy (data-parallel over the 800 flattened (b,t) rows, 100 per core):

  Key identity: sum_v exp(e[t,v] + p[u,v]) = exp(e[t,:]) . exp(p[u,:]),
  so the log-softmax denominator for ALL 5000x1024 logits per core is a
  single [100,1024] x [1024,50] matmul over exp'd projections -- no exp
  over the big tensor.

  setup (per core, on device; pred chain FIRST since it gates M, ep, S):
    tpT  = tanh(predT_slice)           [512, 50]   (ACT)
    pred_b = tpT.T @ WpT + b           [50, 1024]  (PE, psum f32 halves)
    teT  = tanh(encT_slice)            [512, 100]
    enc_p  = teT.T @ WeT               [100, 1024]
    ee, ep = exp(enc_p), exp(pred_b)   (ACT, bf16)
    eeT, epT = transpose(ee), transpose(ep)        (PE transposes)
    S[t,u] = ee[t,:] . ep[u,:]                     (PE, 8 matmuls)
    nlse = -ln(S)                      [100, 50]   (ACT + DVE negate)
    nlse --DMA--> DRAM --DMA--> nlse_rows [128,40] (row-major flatten)
    M_A  = [pred_b ; enc_p[0:50]]      [100, 1024] (SBUF-SBUF DMA)
    M_B  = [pred_b ; enc_p[50:100]]    [100, 1024]
  main loop over 40 row-tiles of 128 rows (row r = t*50 + u), each tile
  split into two 512-vocab halves so every x psum tile is ONE bank
  (8-deep pipeline keeps the PE fed):
    x    = comb_k.T @ M                (PE: one one-hot matmul per half;
           comb has the u one-hot in rows 0-49, t one-hot in rows 50-99)
    tiles 0..HYB-1 (hybrid, so output DMA starts before the nlse round
    trip lands): lse from ACT exp+accum_out per half + ln.
    tiles HYB..: half a: DVE tensor_scalar_add(x, nlse_rows[:,k]);
                 half b: ACT Identity-with-bias. (bf16 out)
    DMA out per PAIR of tiles -> DRAM (sync / gpsimd alternating)

  Output is written bf16 (halves the 20.5MB/core store) and upcast to
  fp32 on the host; bf16 rounding is ~0.4% relative, far inside the
  tolerance.
"""

import numpy as np

import concourse.bass as bass
import concourse.bacc as bacc
import concourse.tile as tile
from concourse import mybir
from concourse.bass_utils import run_bass_kernel_spmd

# Problem shapes (hardcoded per contract).
B, T, U, D, V = 4, 200, 50, 512, 1024
N_CORES = 8
BT = B * T                     # 800 flattened (b,t) rows
TPC = BT // N_CORES            # 100 (b,t) rows per core
ROWS = TPC * U                 # 5000 output rows per core
P = 128
NT = (ROWS + P - 1) // P       # 40 row-tiles per core
HV = 512                       # vocab half (psum-bank limit: 512 f32)
DC = D // P                    # 4 contraction chunks of 128 for D=512
VC = V // P                    # 8 vocab chunks of 128 (transposes / S)
KSPLIT = 19                    # tiles < 19 have all t < 50; tile 19 spans
HYB = 6                        # tiles 0..5 compute lse locally (latency)

f32 = mybir.dt.float32
bf16 = mybir.dt.bfloat16

TRACE = False
LAST_RESULT = None

_CACHE = {}


def _patch_act_tables():
    """Pin Exp/Ln to the one table set containing both, so the activation
    table-load pass never alternates sets mid-kernel."""
    if getattr(bacc, "_joint_act_patch", False):
        return
    orig = bacc.get_activation_tables

    def patched(arch):
        t = dict(orig(arch))
        keep = "natural_log_exp_and_others"
        drop = {mybir.ActivationFunctionType.Exp, mybir.ActivationFunctionType.Ln}
        for name, fns in t.items():
            if name != keep:
                t[name] = set(fns) - drop
        return t

    bacc.get_activation_tables = patched
    bacc._joint_act_patch = True


def _build_indicators():
    """Per-row-tile one-hot stationary operands, shared by all cores.

    comb[k] is [100, 128]: rows 0-49 are the u one-hot (row u has a 1 in
    column c iff u(r)=u for r=128k+c), rows 50-99 the t one-hot relative
    to the M_A/M_B half (t for k<19, t-50 for k>19).  Tile 19 straddles
    the halves and uses separate u19/t19 one-hots the classic way.
    Columns for rows >= ROWS (tail of the last tile) are all-zero.
    """
    comb = np.zeros((NT, TPC, P), dtype=np.float32)
    for k in range(NT):
        if k == KSPLIT:
            continue
        r = np.arange(k * P, min((k + 1) * P, ROWS))
        c = np.arange(len(r))
        off = 50 if k > KSPLIT else 0
        comb[k, (r % U)[c], c] = 1.0
        comb[k, 50 + (r // U)[c] - off, c] = 1.0
    r = np.arange(KSPLIT * P, (KSPLIT + 1) * P)
    c = np.arange(P)
    u19 = np.zeros((U, P), dtype=np.float32)
    t19 = np.zeros((TPC, P), dtype=np.float32)
    u19[r % U, c] = 1.0
    t19[r // U, c] = 1.0
    return np.ascontiguousarray(comb.transpose(1, 0, 2)), u19, t19


def _build_program():
    _patch_act_tables()
    nc = bacc.Bacc("TRN2", target_bir_lowering=False, debug=False,
                   num_devices=N_CORES)

    # encT/predT arrive pre-chunked [128, DC, *] so each partition's DMA
    # read is one contiguous run (128 descriptors, not 512).
    encT = nc.dram_tensor("encT", [P, DC, TPC], f32, kind="ExternalInput")
    predT = nc.dram_tensor("predT", [P, DC, U], f32, kind="ExternalInput")
    wT = nc.dram_tensor("wT", [2 * D, V], bf16, kind="ExternalInput")
    bias = nc.dram_tensor("bias", [V], bf16, kind="ExternalInput")
    comb = nc.dram_tensor("comb", [TPC, NT, P], bf16, kind="ExternalInput")
    u19 = nc.dram_tensor("u19", [U, P], bf16, kind="ExternalInput")
    t19 = nc.dram_tensor("t19", [TPC, P], bf16, kind="ExternalInput")
    ident = nc.dram_tensor("ident", [P, P], bf16, kind="ExternalInput")
    lse_scr = nc.dram_tensor("lse_scr", [NT * P], f32, kind="Internal")
    out = nc.dram_tensor("out", [ROWS, V], bf16, kind="ExternalOutput")

    Act = mybir.ActivationFunctionType
    HALF = (slice(0, HV), slice(HV, V))

    with tile.TileContext(nc) as tc:
        with (
            tc.tile_pool(name="consts", bufs=1) as consts,
            tc.tile_pool(name="psum", bufs=8, space=bass.MemorySpace.PSUM) as psum,
            tc.tile_pool(name="outs", bufs=4) as outs,
            tc.tile_pool(name="scratch", bufs=2) as scratch,
            tc.tile_pool(name="small", bufs=8) as small,
        ):
            # ---- input DMAs, spread over the three issue queues; the
            #      pred-chain inputs (predT, Wp) come first ----
            wt_sb = consts.tile([P, 2 * DC, V], bf16)
            wT_r = wT.ap().rearrange("(c p) v -> p c v", p=P)
            # sync: Wp 0-1, We 0-1, then M assembly + lse write + outputs
            nc.sync.dma_start(out=wt_sb[:, DC + 0, :], in_=wT_r[:, DC + 0, :])
            nc.sync.dma_start(out=wt_sb[:, DC + 1, :], in_=wT_r[:, DC + 1, :])
            nc.sync.dma_start(out=wt_sb[:, 0, :], in_=wT_r[:, 0, :])
            nc.sync.dma_start(out=wt_sb[:, 1, :], in_=wT_r[:, 1, :])
            # scalar: Wp 2-3, We 2-3 (issued before the engine's tanh work)
            nc.scalar.dma_start(out=wt_sb[:, DC + 2, :], in_=wT_r[:, DC + 2, :])
            nc.scalar.dma_start(out=wt_sb[:, DC + 3, :], in_=wT_r[:, DC + 3, :])
            nc.scalar.dma_start(out=wt_sb[:, 2, :], in_=wT_r[:, 2, :])
            nc.scalar.dma_start(out=wt_sb[:, 3, :], in_=wT_r[:, 3, :])
            # gpsimd: activations, bias, ident, indicators
            predT_sb = consts.tile([P, DC, U], f32)
            nc.gpsimd.dma_start(out=predT_sb[:], in_=predT.ap())
            encT_sb = consts.tile([P, DC, TPC], f32)
            nc.gpsimd.dma_start(out=encT_sb[:], in_=encT.ap())
            b_sb = consts.tile([1, V], bf16)
            nc.gpsimd.dma_start(out=b_sb[:], in_=bias.ap().rearrange(
                "(p v) -> p v", p=1))
            ident_sb = consts.tile([P, P], bf16)
            nc.gpsimd.dma_start(out=ident_sb[:], in_=ident.ap())
            comb_sb = consts.tile([TPC, NT, P], bf16)
            nc.gpsimd.dma_start(out=comb_sb[:, :6, :], in_=comb.ap()[:, :6, :])
            u19_sb = consts.tile([U, P], bf16)
            nc.gpsimd.dma_start(out=u19_sb[:], in_=u19.ap())
            t19_sb = consts.tile([TPC, P], bf16)
            nc.gpsimd.dma_start(out=t19_sb[:], in_=t19.ap())
            nc.gpsimd.dma_start(out=comb_sb[:, 6:, :], in_=comb.ap()[:, 6:, :])
            ones_u = consts.tile([1, U], bf16)
            nc.vector.memset(ones_u[:], 1.0)

            # ---- PE warmup: keep the tensor engine continuously busy so
            #      it p-state-ramps to full clock before the real matmuls
            #      (idle gaps reset the ramp; full speed needs ~3us busy) ----
            ones_row = consts.tile([1, P], bf16)
            nc.vector.memset(ones_row[:], 1.0)
            junk_in = consts.tile([1, HV], bf16)
            nc.vector.memset(junk_in[:], 1.0)
            for i in range(16):
                jp = psum.tile([P, HV], f32, tag="x", name=f"junk{i}")
                nc.tensor.matmul(jp[:], ones_row[:], junk_in[:],
                                 start=True, stop=True)

            # ---- pred chain: tanh -> matmul halves (+bias) ----
            tpT = consts.tile([P, DC, U], bf16)
            nc.scalar.activation(tpT[:], predT_sb[:], Act.Tanh)
            teT = consts.tile([P, DC, TPC], bf16)
            nc.scalar.activation(teT[:], encT_sb[:], Act.Tanh)
            # dummy exp: pull the Exp/Ln activation-table load off the
            # critical path (it costs ~1.5us on first use)
            dummy_e = consts.tile([1, U], f32)
            nc.scalar.activation(dummy_e[:], ones_u[:], Act.Exp)

            pred_h = []
            for h in range(2):
                ph = psum.tile([P, HV], f32, tag="x", name=f"pred_ps{h}")
                for c in range(DC):
                    nc.tensor.matmul(ph[:U, :], tpT[:, c, :],
                                     wt_sb[:, DC + c, HALF[h]],
                                     start=(c == 0), stop=False)
                nc.tensor.matmul(ph[:U, :], ones_u[:], b_sb[:, HALF[h]],
                                 start=False, stop=True)
                pred_h.append(ph)
            enc_h = []
            for h in range(2):
                eh = psum.tile([P, HV], f32, tag="x", name=f"enc_ps{h}")
                for c in range(DC):
                    nc.tensor.matmul(eh[:TPC, :], teT[:, c, :],
                                     wt_sb[:, c, HALF[h]],
                                     start=(c == 0), stop=(c == DC - 1))
                enc_h.append(eh)

            # ---- bf16 copies + exp'd projections ----
            pred_sb = consts.tile([U, V], bf16)
            ep = consts.tile([U, V], bf16)
            enc_sb = consts.tile([TPC, V], bf16)
            ee = consts.tile([TPC, V], bf16)
            for h in range(2):
                nc.scalar.activation(pred_sb[:, HALF[h]], pred_h[h][:U, :],
                                     Act.Copy)
                nc.scalar.activation(ep[:, HALF[h]], pred_h[h][:U, :],
                                     Act.Exp)
            # enc copies split by partition (32-aligned): M_A only needs
            # rows 0-49, so rows 0-63 land (and unblock the M_A DMA) first
            for h in range(2):
                nc.vector.tensor_copy(enc_sb[:64, HALF[h]], enc_h[h][:64, :])
            for h in range(2):
                nc.vector.tensor_copy(enc_sb[64:, HALF[h]], enc_h[h][64:TPC, :])
                nc.scalar.activation(ee[:, HALF[h]], enc_h[h][:TPC, :],
                                     Act.Exp)

            # ---- M_A/M_B: stacked moving operands (SBUF-SBUF DMA on the
            #      otherwise-idle sync queue) ----
            m_a = consts.tile([TPC, V], bf16)
            m_b = consts.tile([TPC, V], bf16)
            nc.sync.dma_start(out=m_a[:U, :], in_=pred_sb[:])
            nc.sync.dma_start(out=m_a[U:, :], in_=enc_sb[:50, :])
            nc.sync.dma_start(out=m_b[:U, :], in_=pred_sb[:])
            nc.sync.dma_start(out=m_b[U:, :], in_=enc_sb[50:, :])

            # ---- transposes: epT[v, u], eeT[v, t] (PE, bf16 psum) ----
            epT_ps = psum.tile([P, VC, U], bf16, tag="x")
            epT_sb = consts.tile([P, VC, U], bf16)
            for c in range(VC):
                nc.tensor.transpose(epT_ps[:, c, :],
                                    ep[:, c * P:(c + 1) * P],
                                    ident_sb[:U, :U])
                nc.scalar.activation(epT_sb[:, c, :], epT_ps[:, c, :],
                                     Act.Copy)
            eeT_ps = psum.tile([P, VC, TPC], bf16, tag="x")
            eeT_sb = consts.tile([P, VC, TPC], bf16)
            for c in range(VC):
                nc.tensor.transpose(eeT_ps[:, c, :],
                                    ee[:, c * P:(c + 1) * P],
                                    ident_sb[:TPC, :TPC])
                nc.vector.tensor_copy(eeT_sb[:, c, :], eeT_ps[:, c, :])

            # ---- main loop; S/nlse emitted interleaved so the PE can
            #      work on early (hybrid) tiles while eeT/epT copies land ----
            def emit_x_half(k, h):
                xh = psum.tile([P, HV], f32, tag="x", name=f"x_{k}_{h}")
                if k == KSPLIT:
                    nc.tensor.matmul(xh[:], u19_sb[:], pred_sb[:, HALF[h]],
                                     start=True, stop=False)
                    nc.tensor.matmul(xh[:], t19_sb[:], enc_sb[:, HALF[h]],
                                     start=False, stop=True)
                else:
                    m = m_a if k < KSPLIT else m_b
                    nc.tensor.matmul(xh[:], comb_sb[:, k, :], m[:, HALF[h]],
                                     start=True, stop=True)
                return xh

            o2 = None
            s_emitted = False
            for k in range(NT):
                r0 = k * P
                rows = min(P, ROWS - r0)
                xa = emit_x_half(k, 0)
                xb = emit_x_half(k, 1)
                if k == 2 and not s_emitted:
                    # S[t, u] = ee[t,:] . ep[u,:];  nlse = -ln(S)
                    s_emitted = True
                    s_ps = psum.tile([P, HV], f32, tag="x", name="s_ps")
                    for c in range(VC):
                        nc.tensor.matmul(s_ps[:TPC, :U], eeT_sb[:, c, :],
                                         epT_sb[:, c, :],
                                         start=(c == 0), stop=(c == VC - 1))
                    lse_sb = consts.tile([TPC, U], f32)
                    nc.scalar.activation(lse_sb[:], s_ps[:TPC, :U], Act.Ln)
                    nlse = consts.tile([TPC, U], f32)
                    nc.vector.tensor_scalar_mul(nlse[:], lse_sb[:], -1.0)
                    # flatten nlse[t,u] -> per-row scalars [128, NT] via a
                    # DRAM round trip (row-major (t,u) IS row order)
                    nc.sync.dma_start(
                        out=lse_scr.ap()[:ROWS].rearrange("(t u) -> t u", u=U),
                        in_=nlse[:])
                    nlse_rows = consts.tile([P, NT], f32)
                    nc.sync.dma_start(
                        out=nlse_rows[:],
                        in_=lse_scr.ap().rearrange("(k p) -> p k", p=P))
                if k % 2 == 0:
                    o2 = outs.tile([P, 2, V], bf16)
                oa = o2[:, k % 2, 0:HV]
                ob = o2[:, k % 2, HV:V]
                if k < HYB:
                    # local lse: ACT exp+accum per half -> combine -> ln
                    sums_a = small.tile([P, 1], f32)
                    sums_b = small.tile([P, 1], f32)
                    escr = scratch.tile([P, V], f32)
                    nc.scalar.activation(escr[:, 0:HV], xa[:], Act.Exp,
                                         accum_out=sums_a[:])
                    nc.scalar.activation(escr[:, HV:V], xb[:], Act.Exp,
                                         accum_out=sums_b[:])
                    stot = small.tile([P, 1], f32)
                    nc.vector.scalar_tensor_tensor(
                        stot[:], sums_a[:], 0.0, sums_b[:],
                        mybir.AluOpType.add, mybir.AluOpType.add)
                    lse_h = small.tile([P, 1], f32)
                    nc.scalar.activation(lse_h[:], stot[:], Act.Ln)
                    nc.vector.tensor_scalar_sub(oa[:rows], xa[:rows],
                                                lse_h[:rows])
                    nc.vector.tensor_scalar_sub(ob[:rows], xb[:rows],
                                                lse_h[:rows])
                else:
                    nc.vector.tensor_scalar_add(oa[:rows], xa[:rows],
                                                nlse_rows[:rows, k:k + 1])
                    nc.scalar.activation(ob[:rows], xb[:rows], Act.Identity,
                                         bias=nlse_rows[:rows, k:k + 1])
                # DMA per pair; the straddling tail pair goes per-tile
                eng = (nc.gpsimd, nc.scalar, nc.sync)[(k // 2) % 3]
                if k % 2 == 1 and rows == P:
                    pr = (k - 1) * P
                    eng.dma_start(
                        out=out.ap()[pr:pr + 2 * P, :].rearrange(
                            "(two p) v -> p two v", p=P),
                        in_=o2[:])
                elif k % 2 == 1 or rows < P:
                    eng.dma_start(out=out.ap()[r0:r0 + rows, :],
                                  in_=o2[:rows, k % 2, :])
                    if k % 2 == 1:
                        eng.dma_start(out=out.ap()[r0 - P:r0, :],
                                      in_=o2[:, 0, :])

    nc.compile()
    return nc


def kernel(enc_out, pred_out, W, b):
    global LAST_RESULT
    enc_out = np.asarray(enc_out, dtype=np.float32)
    pred_out = np.asarray(pred_out, dtype=np.float32)
    W = np.asarray(W, dtype=np.float32)
    b = np.asarray(b, dtype=np.float32)

    if "nc" not in _CACHE:
        _CACHE["nc"] = _build_program()
        _CACHE["ind"] = _build_indicators()
    nc = _CACHE["nc"]
    comb, u19, t19 = _CACHE["ind"]

    import ml_dtypes
    wT = np.ascontiguousarray(W.T).astype(ml_dtypes.bfloat16)   # [2D, V]
    enc_flat = enc_out.reshape(BT, D)                 # [800, 512]
    ident = np.eye(P, dtype=ml_dtypes.bfloat16)
    comb_bf = comb.astype(ml_dtypes.bfloat16)
    u19_bf = u19.astype(ml_dtypes.bfloat16)
    t19_bf = t19.astype(ml_dtypes.bfloat16)

    def chunkT(x):  # [n, D] -> [128, DC, n] contiguous
        return np.ascontiguousarray(
            x.T.reshape(DC, P, x.shape[0]).transpose(1, 0, 2))

    in_maps = []
    for c in range(N_CORES):
        bt0 = c * TPC
        b_idx = bt0 // T
        in_maps.append({
            "encT": chunkT(enc_flat[bt0:bt0 + TPC]),
            "predT": chunkT(pred_out[b_idx]),
            "wT": wT,
            "bias": b.astype(ml_dtypes.bfloat16),
            "comb": comb_bf,
            "u19": u19_bf,
            "t19": t19_bf,
            "ident": ident,
        })

    res = run_bass_kernel_spmd(nc, in_maps, core_ids=list(range(N_CORES)),
                               trace=TRACE)
    LAST_RESULT = res
    full = np.concatenate([r["out"] for r in res.results], axis=0)
    return full.reshape(B, T, U, V).astype(np.float32)


# revision 30
# speedup vs baseline: 1.2295x; 1.0791x over previous
"""Trainium2 Bass kernel for the RNN-T JointNetwork problem.

Computes log_softmax(tanh(cat(enc, pred)) @ W.T + b) over the vocab dim
for logits of shape [B=4, T=200, U=50, V=1024], fp32.

Strategy (data-parallel over the 800 flattened (b,t) rows, 100 per core):

  Key identity: sum_v exp(e[t,v] + p[u,v]) = exp(e[t,:]) . exp(p[u,:]),
  so the log-softmax denominator for ALL 5000x1024 logits per core is a
  single [100,1024] x [1024,50] matmul over exp'd projections -- no exp
  over the big tensor.

  setup (per core, on device; pred chain FIRST since it gates M, ep, S):
    tpT  = tanh(predT_slice)           [512, 50]   (ACT)
    pred_b = tpT.T @ WpT + b           [50, 1024]  (PE, psum f32 halves)
    teT  = tanh(encT_slice)            [512, 100]
    enc_p  = teT.T @ WeT               [100, 1024]
    ee, ep = exp(enc_p), exp(pred_b)   (ACT, bf16)
    eeT, epT = transpose(ee), transpose(ep)        (PE transposes)
    S[t,u] = ee[t,:] . ep[u,:]                     (PE, 8 matmuls)
    nlse = -ln(S)                      [100, 50]   (ACT + DVE negate)
    nlse --DMA--> DRAM --DMA--> nlse_rows [128,40] (row-major flatten)
    M_A  = [pred_b ; enc_p[0:50]]      [100, 1024] (SBUF-SBUF DMA)
    M_B  = [pred_b ; enc_p[50:100]]    [100, 1024]
  main loop over 40 row-tiles of 128 rows (row r = t*50 + u), each tile
  split into two 512-vocab halves so every x psum tile is ONE bank
  (8-deep pipeline keeps the PE fed):
    x    = comb_k.T @ M                (PE: one one-hot matmul per half;
           comb has the u one-hot in rows 0-49, t one-hot in rows 50-99)
    tiles 0..HYB-1 (hybrid, so output DMA starts before the nlse round
    trip lands): lse from ACT exp+accum_out per half + ln.
    tiles HYB..: half a: DVE tensor_scalar_add(x, nlse_rows[:,k]);
                 half b: ACT Identity-with-bias. (bf16 out)
    DMA out per PAIR of tiles -> DRAM (sync / gpsimd alternating)

  Output is written bf16 (halves the 20.5MB/core store) and upcast to
  fp32 on the host; bf16 rounding is ~0.4% relative, far inside the
  tolerance.
"""

import numpy as np

import concourse.bass as bass
import concourse.bacc as bacc
import concourse.tile as tile
from concourse import mybir
from concourse.bass_utils import run_bass_kernel_spmd

# Problem shapes (hardcoded per contract).
B, T, U, D, V = 4, 200, 50, 512, 1024
N_CORES = 8
BT = B * T                     # 800 flattened (b,t) rows
TPC = BT // N_CORES            # 100 (b,t) rows per core
ROWS = TPC * U                 # 5000 output rows per core
P = 128
NT = (ROWS + P - 1) // P       # 40 row-tiles per core
HV = 512                       # vocab half (psum-bank limit: 512 f32)
DC = D // P                    # 4 contraction chunks of 128 for D=512
VC = V // P                    # 8 vocab chunks of 128 (transposes / S)
KSPLIT = 19                    # tiles < 19 have all t < 50; tile 19 spans
HYB = 6                        # tiles 0..5 compute lse locally (latency)

f32 = mybir.dt.float32
bf16 = mybir.dt.bfloat16

TRACE = False
LAST_RESULT = None

_CACHE = {}


def _patch_act_tables():
    """Pin Exp/Ln to the one table set containing both, so the activation
    table-load pass never alternates sets mid-kernel."""
    if getattr(bacc, "_joint_act_patch", False):
        return
    orig = bacc.get_activation_tables

    def patched(arch):
        t = dict(orig(arch))
        keep = "natural_log_exp_and_others"
        drop = {mybir.ActivationFunctionType.Exp, mybir.ActivationFunctionType.Ln}
        for name, fns in t.items():
            if name != keep:
                t[name] = set(fns) - drop
        return t

    bacc.get_activation_tables = patched
    bacc._joint_act_patch = True


def _build_indicators():
    """Per-row-tile one-hot stationary operands, shared by all cores.

    comb[k] is [100, 128]: rows 0-49 are the u one-hot (row u has a 1 in
    column c iff u(r)=u for r=128k+c), rows 50-99 the t one-hot relative
    to the M_A/M_B half (t for k<19, t-50 for k>19).  Tile 19 straddles
    the halves and uses separate u19/t19 one-hots the classic way.
    Columns for rows >= ROWS (tail of the last tile) are all-zero.
    """
    comb = np.zeros((NT, TPC, P), dtype=np.float32)
    for k in range(NT):
        if k == KSPLIT:
            continue
        r = np.arange(k * P, min((k + 1) * P, ROWS))
        c = np.arange(len(r))
        off = 50 if k > KSPLIT else 0
        comb[k, (r % U)[c], c] = 1.0
        comb[k, 50 + (r // U)[c] - off, c] = 1.0
    r = np.arange(KSPLIT * P, (KSPLIT + 1) * P)
    c = np.arange(P)
    u19 = np.zeros((U, P), dtype=np.float32)
    t19 = np.zeros((TPC, P), dtype=np.float32)
    u19[r % U, c] = 1.0
    t19[r // U, c] = 1.0
    return np.ascontiguousarray(comb.transpose(1, 0, 2)), u19, t19


def _build_program():
    _patch_act_tables()
    nc = bacc.Bacc("TRN2", target_bir_lowering=False, debug=False,
                   num_devices=N_CORES)

    # encT/predT arrive pre-chunked [128, DC, *] so each partition's DMA
    # read is one contiguous run (128 descriptors, not 512).
    encT = nc.dram_tensor("encT", [P, DC, TPC], f32, kind="ExternalInput")
    predT = nc.dram_tensor("predT", [P, DC, U], f32, kind="ExternalInput")
    wT = nc.dram_tensor("wT", [2 * D, V], bf16, kind="ExternalInput")
    bias = nc.dram_tensor("bias", [V], bf16, kind="ExternalInput")
    comb = nc.dram_tensor("comb", [TPC, NT, P], bf16, kind="ExternalInput")
    u19 = nc.dram_tensor("u19", [U, P], bf16, kind="ExternalInput")
    t19 = nc.dram_tensor("t19", [TPC, P], bf16, kind="ExternalInput")
    ident = nc.dram_tensor("ident", [P, P], bf16, kind="ExternalInput")
    lse_scr = nc.dram_tensor("lse_scr", [NT * P], f32, kind="Internal")
    out = nc.dram_tensor("out", [ROWS, V], bf16, kind="ExternalOutput")

    Act = mybir.ActivationFunctionType
    HALF = (slice(0, HV), slice(HV, V))

    with tile.TileContext(nc) as tc:
        with (
            tc.tile_pool(name="consts", bufs=1) as consts,
            tc.tile_pool(name="psum", bufs=8, space=bass.MemorySpace.PSUM) as psum,
            tc.tile_pool(name="outs", bufs=8) as outs,
            tc.tile_pool(name="scratch", bufs=2) as scratch,
            tc.tile_pool(name="small", bufs=8) as small,
        ):
            # ---- input DMAs, spread over the three issue queues; the
            #      pred-chain inputs (predT, Wp) come first ----
            wt_sb = consts.tile([P, 2 * DC, V], bf16)
            wT_r = wT.ap().rearrange("(c p) v -> p c v", p=P)
            # sync: Wp 0-1, We 0-1, then M assembly + lse write + outputs
            nc.sync.dma_start(out=wt_sb[:, DC + 0, :], in_=wT_r[:, DC + 0, :])
            nc.sync.dma_start(out=wt_sb[:, DC + 1, :], in_=wT_r[:, DC + 1, :])
            nc.sync.dma_start(out=wt_sb[:, 0, :], in_=wT_r[:, 0, :])
            nc.sync.dma_start(out=wt_sb[:, 1, :], in_=wT_r[:, 1, :])
            # scalar: Wp 2-3, We 2-3 (issued before the engine's tanh work)
            nc.scalar.dma_start(out=wt_sb[:, DC + 2, :], in_=wT_r[:, DC + 2, :])
            nc.scalar.dma_start(out=wt_sb[:, DC + 3, :], in_=wT_r[:, DC + 3, :])
            nc.scalar.dma_start(out=wt_sb[:, 2, :], in_=wT_r[:, 2, :])
            nc.scalar.dma_start(out=wt_sb[:, 3, :], in_=wT_r[:, 3, :])
            # gpsimd: activations, bias, ident, indicators
            predT_sb = consts.tile([P, DC, U], f32)
            nc.gpsimd.dma_start(out=predT_sb[:], in_=predT.ap())
            encT_sb = consts.tile([P, DC, TPC], f32)
            nc.gpsimd.dma_start(out=encT_sb[:], in_=encT.ap())
            b_sb = consts.tile([1, V], bf16)
            nc.gpsimd.dma_start(out=b_sb[:], in_=bias.ap().rearrange(
                "(p v) -> p v", p=1))
            ident_sb = consts.tile([P, P], bf16)
            nc.gpsimd.dma_start(out=ident_sb[:], in_=ident.ap())
            comb_sb = consts.tile([TPC, NT, P], bf16)
            nc.gpsimd.dma_start(out=comb_sb[:, :6, :], in_=comb.ap()[:, :6, :])
            u19_sb = consts.tile([U, P], bf16)
            nc.gpsimd.dma_start(out=u19_sb[:], in_=u19.ap())
            t19_sb = consts.tile([TPC, P], bf16)
            nc.gpsimd.dma_start(out=t19_sb[:], in_=t19.ap())
            nc.gpsimd.dma_start(out=comb_sb[:, 6:, :], in_=comb.ap()[:, 6:, :])
            ones_u = consts.tile([1, U], bf16)
            nc.vector.memset(ones_u[:], 1.0)

            # ---- PE warmup: keep the tensor engine continuously busy so
            #      it p-state-ramps to full clock before the real matmuls
            #      (idle gaps reset the ramp; full speed needs ~3us busy) ----
            ones_row = consts.tile([1, P], bf16)
            nc.vector.memset(ones_row[:], 1.0)
            junk_in = consts.tile([1, HV], bf16)
            nc.vector.memset(junk_in[:], 1.0)
            for i in range(10):
                jp = psum.tile([P, HV], f32, tag="x", name=f"junk{i}")
                nc.tensor.matmul(jp[:], ones_row[:], junk_in[:],
                                 start=True, stop=True)

            # ---- pred chain: tanh -> matmul halves (+bias) ----
            tpT = consts.tile([P, DC, U], bf16)
            nc.scalar.activation(tpT[:], predT_sb[:], Act.Tanh)
            teT = consts.tile([P, DC, TPC], bf16)
            nc.scalar.activation(teT[:], encT_sb[:], Act.Tanh)
            # dummy exp: pull the Exp/Ln activation-table load off the
            # critical path (it costs ~1.5us on first use)
            dummy_e = consts.tile([1, U], f32)
            nc.scalar.activation(dummy_e[:], ones_u[:], Act.Exp)

            pred_h = []
            for h in range(2):
                ph = psum.tile([P, HV], f32, tag="x", name=f"pred_ps{h}")
                for c in range(DC):
                    nc.tensor.matmul(ph[:U, :], tpT[:, c, :],
                                     wt_sb[:, DC + c, HALF[h]],
                                     start=(c == 0), stop=False)
                nc.tensor.matmul(ph[:U, :], ones_u[:], b_sb[:, HALF[h]],
                                 start=False, stop=True)
                pred_h.append(ph)
            enc_h = []
            for h in range(2):
                eh = psum.tile([P, HV], f32, tag="x", name=f"enc_ps{h}")
                for c in range(DC):
                    nc.tensor.matmul(eh[:TPC, :], teT[:, c, :],
                                     wt_sb[:, c, HALF[h]],
                                     start=(c == 0), stop=(c == DC - 1))
                enc_h.append(eh)

            # ---- bf16 copies + exp'd projections ----
            pred_sb = consts.tile([U, V], bf16)
            ep = consts.tile([U, V], bf16)
            enc_sb = consts.tile([TPC, V], bf16)
            ee = consts.tile([TPC, V], bf16)
            for h in range(2):
                nc.scalar.activation(pred_sb[:, HALF[h]], pred_h[h][:U, :],
                                     Act.Copy)
                nc.scalar.activation(ep[:, HALF[h]], pred_h[h][:U, :],
                                     Act.Exp)
            # enc copies split by partition (32-aligned): M_A only needs
            # rows 0-49, so rows 0-63 land (and unblock the M_A DMA) first
            for h in range(2):
                nc.vector.tensor_copy(enc_sb[:64, HALF[h]], enc_h[h][:64, :])
            for h in range(2):
                nc.vector.tensor_copy(enc_sb[64:, HALF[h]], enc_h[h][64:TPC, :])
                nc.scalar.activation(ee[:, HALF[h]], enc_h[h][:TPC, :],
                                     Act.Exp)

            # ---- M_A/M_B: stacked moving operands (SBUF-SBUF DMA on the
            #      otherwise-idle sync queue) ----
            m_a = consts.tile([TPC, V], bf16)
            m_b = consts.tile([TPC, V], bf16)
            nc.sync.dma_start(out=m_a[:U, :], in_=pred_sb[:])
            nc.sync.dma_start(out=m_a[U:, :], in_=enc_sb[:50, :])
            nc.sync.dma_start(out=m_b[:U, :], in_=pred_sb[:])
            nc.sync.dma_start(out=m_b[U:, :], in_=enc_sb[50:, :])

            # ---- transposes: epT[v, u], eeT[v, t] (PE, bf16 psum) ----
            epT_ps = psum.tile([P, VC, U], bf16, tag="x")
            epT_sb = consts.tile([P, VC, U], bf16)
            for c in range(VC):
                nc.tensor.transpose(epT_ps[:, c, :],
                                    ep[:, c * P:(c + 1) * P],
                                    ident_sb[:U, :U])
                nc.scalar.activation(epT_sb[:, c, :], epT_ps[:, c, :],
                                     Act.Copy)
            eeT_ps = psum.tile([P, VC, TPC], bf16, tag="x")
            eeT_sb = consts.tile([P, VC, TPC], bf16)
            for c in range(VC):
                nc.tensor.transpose(eeT_ps[:, c, :],
                                    ee[:, c * P:(c + 1) * P],
                                    ident_sb[:TPC, :TPC])
                nc.vector.tensor_copy(eeT_sb[:, c, :], eeT_ps[:, c, :])

            # ---- main loop; S/nlse emitted interleaved so the PE can
            #      work on early (hybrid) tiles while eeT/epT copies land ----
            def emit_x_half(k, h):
                xh = psum.tile([P, HV], f32, tag="x", name=f"x_{k}_{h}")
                if k == KSPLIT:
                    nc.tensor.matmul(xh[:], u19_sb[:], pred_sb[:, HALF[h]],
                                     start=True, stop=False)
                    nc.tensor.matmul(xh[:], t19_sb[:], enc_sb[:, HALF[h]],
                                     start=False, stop=True)
                else:
                    m = m_a if k < KSPLIT else m_b
                    nc.tensor.matmul(xh[:], comb_sb[:, k, :], m[:, HALF[h]],
                                     start=True, stop=True)
                return xh

            o2 = None
            s_emitted = False
            for k in range(NT):
                r0 = k * P
                rows = min(P, ROWS - r0)
                xa = emit_x_half(k, 0)
                xb = emit_x_half(k, 1)
                if k == 2 and not s_emitted:
                    # S[t, u] = ee[t,:] . ep[u,:];  nlse = -ln(S)
                    s_emitted = True
                    s_ps = psum.tile([P, HV], f32, tag="x", name="s_ps")
                    for c in range(VC):
                        nc.tensor.matmul(s_ps[:TPC, :U], eeT_sb[:, c, :],
                                         epT_sb[:, c, :],
                                         start=(c == 0), stop=(c == VC - 1))
                    lse_sb = consts.tile([TPC, U], f32)
                    nc.scalar.activation(lse_sb[:], s_ps[:TPC, :U], Act.Ln)
                    nlse = consts.tile([TPC, U], f32)
                    nc.vector.tensor_scalar_mul(nlse[:], lse_sb[:], -1.0)
                    # flatten nlse[t,u] -> per-row scalars [128, NT] via a
                    # DRAM round trip (row-major (t,u) IS row order)
                    nc.sync.dma_start(
                        out=lse_scr.ap()[:ROWS].rearrange("(t u) -> t u", u=U),
                        in_=nlse[:])
                    nlse_rows = consts.tile([P, NT], f32)
                    nc.sync.dma_start(
                        out=nlse_rows[:],
                        in_=lse_scr.ap().rearrange("(k p) -> p k", p=P))
                if k % 2 == 0:
                    o2 = outs.tile([P, 2, V], bf16)
                oa = o2[:, k % 2, 0:HV]
                ob = o2[:, k % 2, HV:V]
                if k < HYB:
                    # local lse: ACT exp+accum per half -> combine -> ln
                    sums_a = small.tile([P, 1], f32)
                    sums_b = small.tile([P, 1], f32)
                    escr = scratch.tile([P, V], f32)
                    nc.scalar.activation(escr[:, 0:HV], xa[:], Act.Exp,
                                         accum_out=sums_a[:])
                    nc.scalar.activation(escr[:, HV:V], xb[:], Act.Exp,
                                         accum_out=sums_b[:])
                    stot = small.tile([P, 1], f32)
                    nc.vector.scalar_tensor_tensor(
                        stot[:], sums_a[:], 0.0, sums_b[:],
                        mybir.AluOpType.add, mybir.AluOpType.add)
                    lse_h = small.tile([P, 1], f32)
                    nc.scalar.activation(lse_h[:], stot[:], Act.Ln)
                    nc.vector.tensor_scalar_sub(oa[:rows], xa[:rows],
                                                lse_h[:rows])
                    nc.vector.tensor_scalar_sub(ob[:rows], xb[:rows],
                                                lse_h[:rows])
                else:
                    nc.vector.tensor_scalar_add(oa[:rows], xa[:rows],
                                                nlse_rows[:rows, k:k + 1])
                    nc.scalar.activation(ob[:rows], xb[:rows], Act.Identity,
                                         bias=nlse_rows[:rows, k:k + 1])
                # DMA per pair; the straddling tail pair goes per-tile
                eng = (nc.gpsimd, nc.sync)[(k // 2) % 2]
                if k % 2 == 1 and rows == P:
                    pr = (k - 1) * P
                    eng.dma_start(
                        out=out.ap()[pr:pr + 2 * P, :].rearrange(
                            "(two p) v -> p two v", p=P),
                        in_=o2[:])
                elif k % 2 == 1 or rows < P:
                    eng.dma_start(out=out.ap()[r0:r0 + rows, :],
                                  in_=o2[:rows, k % 2, :])
                    if k % 2 == 1:
                        eng.dma_start(out=out.ap()[r0 - P:r0, :],
                                      in_=o2[:, 0, :])

    nc.compile()
    return nc


def kernel(enc_out, pred_out, W, b):
    global LAST_RESULT
    enc_out = np.asarray(enc_out, dtype=np.float32)
    pred_out = np.asarray(pred_out, dtype=np.float32)
    W = np.asarray(W, dtype=np.float32)
    b = np.asarray(b, dtype=np.float32)

    if "nc" not in _CACHE:
        _CACHE["nc"] = _build_program()
        _CACHE["ind"] = _build_indicators()
    nc = _CACHE["nc"]
    comb, u19, t19 = _CACHE["ind"]

    import ml_dtypes
    wT = np.ascontiguousarray(W.T).astype(ml_dtypes.bfloat16)   # [2D, V]
    enc_flat = enc_out.reshape(BT, D)                 # [800, 512]
    ident = np.eye(P, dtype=ml_dtypes.bfloat16)
    comb_bf = comb.astype(ml_dtypes.bfloat16)
    u19_bf = u19.astype(ml_dtypes.bfloat16)
    t19_bf = t19.astype(ml_dtypes.bfloat16)

    def chunkT(x):  # [n, D] -> [128, DC, n] contiguous
        return np.ascontiguousarray(
            x.T.reshape(DC, P, x.shape[0]).transpose(1, 0, 2))

    in_maps = []
    for c in range(N_CORES):
        bt0 = c * TPC
        b_idx = bt0 // T
        in_maps.append({
            "encT": chunkT(enc_flat[bt0:bt0 + TPC]),
            "predT": chunkT(pred_out[b_idx]),
            "wT": wT,
            "bias": b.astype(ml_dtypes.bfloat16),
            "comb": comb_bf,
            "u19": u19_bf,
            "t19": t19_bf,
            "ident": ident,
        })

    res = run_bass_kernel_spmd(nc, in_maps, core_ids=list(range(N_CORES)),
                               trace=TRACE)
    LAST_RESULT = res
    full = np.concatenate([r["out"] for r in res.results], axis=0)
    return full.reshape(B, T, U, V).astype(np.float32)
